# revision 5
# baseline (speedup 1.0000x reference)
"""Bass kernel for nn_Attn_1898375545663 on 8 TRN2 NeuronCores.

Reference (single device):
    energies[b, l] = sum_h hidden[h, b] * encoder_outputs[l, b, h]   # [B, L]
    attn = softmax(energies, axis=1)                                 # [B, L]
    return attn[:, None, :]                                          # [B, 1, L]

Shapes: L=4096, B=32, H=1024, fp32. encoder_outputs is 512 MB -> memory bound.

Sharding: pure data parallel over batch. Each of the 8 cores gets 4 batches
(encoder shard [4096, 4, 1024] = 64 MB); no collectives.

Per-core kernel:
  - hidden shard arrives as one row [1, 4*1024]; gpsimd.partition_broadcast
    replicates it to [128, 4096] so the DVE can use it per-partition.
  - Main loop: DMA encoder tiles [128 l-rows, TG tiles, 1024 h] (2 MB per
    dma_start for DMA efficiency), then one fused DVE affine_mul_reduce per
    (batch, l-tile): the elementwise product goes to a stride-0 dummy, the
    free-axis (h) sum lands in en[:, c] (c = b*32 + t). One DVE pass/element.
  - Softmax: global max over all 4 batches (any per-batch constant is exact
    for softmax; the global max keeps exp in range), exp on ScalarE,
    per-batch sums via a PE matmul against a ones vector, reciprocal, then a
    PE transpose so each (b, t) partition holds 128 contiguous l values,
    scale by 1/sum, one output DMA.
"""

import numpy as np

from concourse import bacc, mybir, tile
from concourse.bass_isa import ReduceOp
from concourse.bass_utils import run_bass_kernel_spmd
from concourse.masks import make_identity

L, B, H = 4096, 32, 1024
NCORES = 8
BS = B // NCORES          # 4 batches per core
P = 128                   # partitions / l-tile height
NT = L // P               # 32 l-tiles per batch
TG = 4                    # l-tiles per DMA group (2 MB per dma_start)
NC_COLS = BS * NT         # 128 energy columns per core
F32 = mybir.dt.float32

_cached = {}


def main_loop(nc, inp, enc, hidb, en, dummy, order="b"):
    pairs = (
        [(b, g) for b in range(BS) for g in range(NT // TG)]
        if order == "b"
        else [(b, g) for g in range(NT // TG) for b in range(BS)]
    )
    for b, g in pairs:
        if True:
            tile_in = inp.tile([P, TG, H], F32)
            src = enc[g * TG * P : (g + 1) * TG * P, b : b + 1, :]
            src = src.rearrange("(t p) o h -> p t (o h)", p=P)
            # alternate the issuing engine: sync and scalar HWDGE rings run
            # in parallel; one ring alone tops out ~20 GB/s below HBM rate
            if (g * BS + b) % 2 == 1 if order == "g" else g % 2 == 1:
                nc.scalar.dma_start(tile_in[:], src)
            else:
                nc.sync.dma_start(tile_in[:], src)
            for t in range(TG):
                c = b * NT + g * TG + t
                nc.vector.affine_mul_reduce(
                    out=dummy.broadcast_to((P, H)),
                    accum_out=en[:, c : c + 1],
                    in0=tile_in[:, t, :],
                    in1=hidb[:, b * H : (b + 1) * H],
                    scale=1.0,
                    bias=0.0,
                )


def main_loop_lblock(nc, inp, enc, hidb, en, dummy):
    # One DMA per 128-row l-block covering ALL 4 batches: the source region
    # enc[g*128:(g+1)*128, :, :] is fully contiguous in DRAM (2 MiB), so each
    # partition reads one 16 KiB contiguous chunk — sequential HBM access,
    # 128 descriptors per dma_start (vs 512 x 4 KiB strided in the b-major
    # layout).
    for g in range(NT):
        tile_in = inp.tile([P, BS * H], F32)
        src = enc[g * P : (g + 1) * P, :, :].rearrange("p b h -> p (b h)")
        if g % 2 == 1:
            nc.scalar.dma_start(tile_in[:], src)
        else:
            nc.sync.dma_start(tile_in[:], src)
        for b in range(BS):
            c = b * NT + g
            nc.vector.affine_mul_reduce(
                out=dummy.broadcast_to((P, H)),
                accum_out=en[:, c : c + 1],
                in0=tile_in[:, b * H : (b + 1) * H],
                in1=hidb[:, b * H : (b + 1) * H],
                scale=1.0,
                bias=0.0,
            )


def softmax_out(nc, work, psum, en, gmat_sb, iden, ones, out_ext):
    # softmax over l (per batch); en[:, c] holds e(l = t*128 + p), c = b*32+t
    m1 = work.tile([P, 1], F32)
    nc.vector.tensor_reduce(
        out=m1[:], in_=en[:], axis=mybir.AxisListType.X, op=mybir.AluOpType.max
    )
    mx = work.tile([P, 1], F32)
    nc.gpsimd.partition_all_reduce(mx[:], m1[:], P, ReduceOp.max)
    negm = work.tile([P, 1], F32)
    nc.scalar.mul(negm[:], mx[:], -1.0)

    p_all = work.tile([P, NC_COLS], F32)
    nc.scalar.activation(
        p_all[:],
        en[:],
        mybir.ActivationFunctionType.Exp,
        bias=negm[:],
        scale=1.0,
    )

    s3 = work.tile([P, BS], F32)
    nc.vector.tensor_reduce(
        out=s3[:],
        in_=p_all[:].rearrange("p (b t) -> p b t", b=BS),
        axis=mybir.AxisListType.X,
        op=mybir.AluOpType.add,
    )
    s_ps = psum.tile([BS, 1], F32)
    nc.tensor.matmul(s_ps[:], s3[:], ones[:], start=True, stop=True)
    r_sb = work.tile([BS, 1], F32)
    nc.vector.reciprocal(r_sb[:], s_ps[:])

    rb_ps = psum.tile([P, 1], F32)
    nc.tensor.matmul(rb_ps[:], gmat_sb[:], r_sb[:], start=True, stop=True)
    rb_sb = work.tile([P, 1], F32)
    nc.scalar.copy(rb_sb[:], rb_ps[:])

    t_ps = psum.tile([P, P], F32)
    nc.tensor.transpose(t_ps[:], p_all[:], iden[:])
    attn_sb = work.tile([P, P], F32)
    nc.vector.tensor_scalar(
        out=attn_sb[:],
        in0=t_ps[:],
        scalar1=rb_sb[:],
        scalar2=None,
        op0=mybir.AluOpType.mult,
    )
    nc.sync.dma_start(out_ext[:], attn_sb[:])


def build_nc(repeat=1, use_for_i=False, order="b", variant="lblock", bufs=8):
    nc = bacc.Bacc(trn_type="TRN2")

    enc = nc.declare_dram_parameter("enc", [L, BS, H], F32, isOutput=False)
    hid = nc.declare_dram_parameter("hid", [1, BS * H], F32, isOutput=False)
    gmat = nc.declare_dram_parameter("gmat", [BS, P], F32, isOutput=False)
    out_ext = nc.declare_dram_parameter("out", [NC_COLS, P], F32, isOutput=True)

    with tile.TileContext(nc) as tc:
        with (
            tc.tile_pool(name="consts", bufs=1) as consts,
            tc.tile_pool(name="inp", bufs=bufs) as inp,
            tc.tile_pool(name="work", bufs=1) as work,
            tc.tile_pool(name="psum", bufs=1, space="PSUM") as psum,
        ):
            hid_row = consts.tile([1, BS * H], F32)
            nc.sync.dma_start(hid_row[:], hid[:])
            gmat_sb = consts.tile([BS, P], F32)
            nc.sync.dma_start(gmat_sb[:], gmat[:])
            iden = consts.tile([P, P], F32)
            make_identity(nc, iden[:])
            ones = consts.tile([P, 1], F32)
            nc.gpsimd.memset(ones[:], 1.0)

            hidb = consts.tile([P, BS * H], F32)
            nc.gpsimd.partition_broadcast(hidb[:], hid_row[:], P)

            en = work.tile([P, NC_COLS], F32)
            dummy = work.tile([P, 1], F32)
            if repeat == 0:
                nc.gpsimd.memset(en[:], 0.0)

            def body():
                if variant == "lblock":
                    main_loop_lblock(nc, inp, enc, hidb, en, dummy)
                else:
                    main_loop(nc, inp, enc, hidb, en, dummy, order)

            if use_for_i and repeat > 1:
                with tc.For_i(0, repeat, 1):
                    body()
            else:
                for _rep in range(repeat):
                    body()

            softmax_out(nc, work, psum, en, gmat_sb, iden, ones, out_ext)

    nc.compile()
    return nc


def make_in_maps(hidden, encoder_outputs):
    hidden = np.ascontiguousarray(np.asarray(hidden, dtype=np.float32))
    enc = np.ascontiguousarray(np.asarray(encoder_outputs, dtype=np.float32))
    assert hidden.shape == (H, B) and enc.shape == (L, B, H)

    gmat = np.zeros((BS, P), np.float32)
    for b in range(BS):
        gmat[b, b * NT : (b + 1) * NT] = 1.0

    in_maps = []
    for c in range(NCORES):
        bsl = slice(c * BS, (c + 1) * BS)
        in_maps.append(
            {
                "enc": np.ascontiguousarray(enc[:, bsl, :]),
                "hid": np.ascontiguousarray(hidden[:, bsl].T.reshape(1, BS * H)),
                "gmat": gmat,
            }
        )
    return in_maps


def _get_nc():
    if "nc" not in _cached:
        _cached["nc"] = build_nc()
    return _cached["nc"]


def kernel(hidden, encoder_outputs, **kwargs):
    in_maps = make_in_maps(hidden, encoder_outputs)
    nc = _get_nc()
    res = run_bass_kernel_spmd(nc, in_maps, core_ids=list(range(NCORES)))
    outs = [res.results[i]["out"].reshape(BS, 1, L) for i in range(NCORES)]
    return np.concatenate(outs, axis=0)



# revision 11
# speedup vs baseline: 1.2757x; 1.2757x over previous
"""Bass kernel for nn_Attn_1898375545663 on 8 TRN2 NeuronCores.

Reference (single device):
    energies[b, l] = sum_h hidden[h, b] * encoder_outputs[l, b, h]   # [B, L]
    attn = softmax(energies, axis=1)                                 # [B, L]
    return attn[:, None, :]                                          # [B, 1, L]

Shapes: L=4096, B=32, H=1024, fp32. encoder_outputs is 512 MB -> memory bound.

Sharding: pure data parallel over batch. Each of the 8 cores gets 4 batches
(encoder shard [4096, 4, 1024] = 64 MB); no collectives.

Per-core kernel:
  - hidden shard arrives as one row [1, 4*1024]; gpsimd.partition_broadcast
    replicates it to [128, 4096] so the DVE can use it per-partition.
  - Main loop: DMA encoder tiles [128 l-rows, TG tiles, 1024 h] (2 MB per
    dma_start for DMA efficiency), then one fused DVE affine_mul_reduce per
    (batch, l-tile): the elementwise product goes to a stride-0 dummy, the
    free-axis (h) sum lands in en[:, c] (c = b*32 + t). One DVE pass/element.
  - Softmax: global max over all 4 batches (any per-batch constant is exact
    for softmax; the global max keeps exp in range), exp on ScalarE,
    per-batch sums via a PE matmul against a ones vector, reciprocal, then a
    PE transpose so each (b, t) partition holds 128 contiguous l values,
    scale by 1/sum, one output DMA.
"""

import numpy as np

from concourse import bacc, mybir, tile
from concourse.bass_isa import ReduceOp
from concourse.bass_utils import run_bass_kernel_spmd
from concourse.masks import make_identity

L, B, H = 4096, 32, 1024
NCORES = 8
BS = B // NCORES          # 4 batches per core
P = 128                   # partitions / l-tile height
NT = L // P               # 32 l-tiles per batch
TG = 4                    # l-tiles per DMA group (2 MB per dma_start)
NC_COLS = BS * NT         # 128 energy columns per core
F32 = mybir.dt.float32

_cached = {}


def main_loop(nc, inp, enc, hidb, en, dummy, order="b"):
    pairs = (
        [(b, g) for b in range(BS) for g in range(NT // TG)]
        if order == "b"
        else [(b, g) for g in range(NT // TG) for b in range(BS)]
    )
    for b, g in pairs:
        if True:
            tile_in = inp.tile([P, TG, H], F32)
            src = enc[g * TG * P : (g + 1) * TG * P, b : b + 1, :]
            src = src.rearrange("(t p) o h -> p t (o h)", p=P)
            # alternate the issuing engine: sync and scalar HWDGE rings run
            # in parallel; one ring alone tops out ~20 GB/s below HBM rate
            if (g * BS + b) % 2 == 1 if order == "g" else g % 2 == 1:
                nc.scalar.dma_start(tile_in[:], src)
            else:
                nc.sync.dma_start(tile_in[:], src)
            for t in range(TG):
                c = b * NT + g * TG + t
                nc.vector.affine_mul_reduce(
                    out=dummy.broadcast_to((P, H)),
                    accum_out=en[:, c : c + 1],
                    in0=tile_in[:, t, :],
                    in1=hidb[:, b * H : (b + 1) * H],
                    scale=1.0,
                    bias=0.0,
                )


def main_loop_lblock(
    nc, inp, enc, hidb, en, dummy, do_dma=True, do_dve=True, fake=None
):
    # One DMA per 128-row l-block covering ALL 4 batches: the source region
    # enc[g*128:(g+1)*128, :, :] is fully contiguous in DRAM (2 MiB), so each
    # partition reads one 16 KiB contiguous chunk — sequential HBM access,
    # 128 descriptors per dma_start (vs 512 x 4 KiB strided in the b-major
    # layout).
    for g in range(NT):
        if do_dma:
            tile_in = inp.tile([P, BS * H], F32)
            src = enc[g * P : (g + 1) * P, :, :].rearrange("p b h -> p (b h)")
            if g % 2 == 1:
                nc.scalar.dma_start(tile_in[:], src)
            else:
                nc.sync.dma_start(tile_in[:], src)
        else:
            tile_in = fake
        if not do_dve:
            continue
        for b in range(BS):
            c = b * NT + g
            nc.vector.affine_mul_reduce(
                out=dummy.broadcast_to((P, H)),
                accum_out=en[:, c : c + 1],
                in0=tile_in[:, b * H : (b + 1) * H],
                in1=hidb[:, b * H : (b + 1) * H],
                scale=1.0,
                bias=0.0,
            )


def softmax_out(nc, work, psum, en, gmat_sb, iden, ones, out_ext):
    # softmax over l (per batch); en[:, c] holds e(l = t*128 + p), c = b*32+t
    m1 = work.tile([P, 1], F32)
    nc.vector.tensor_reduce(
        out=m1[:], in_=en[:], axis=mybir.AxisListType.X, op=mybir.AluOpType.max
    )
    mx = work.tile([P, 1], F32)
    nc.gpsimd.partition_all_reduce(mx[:], m1[:], P, ReduceOp.max)
    negm = work.tile([P, 1], F32)
    nc.scalar.mul(negm[:], mx[:], -1.0)

    p_all = work.tile([P, NC_COLS], F32)
    nc.scalar.activation(
        p_all[:],
        en[:],
        mybir.ActivationFunctionType.Exp,
        bias=negm[:],
        scale=1.0,
    )

    s3 = work.tile([P, BS], F32)
    nc.vector.tensor_reduce(
        out=s3[:],
        in_=p_all[:].rearrange("p (b t) -> p b t", b=BS),
        axis=mybir.AxisListType.X,
        op=mybir.AluOpType.add,
    )
    s_ps = psum.tile([BS, 1], F32)
    nc.tensor.matmul(s_ps[:], s3[:], ones[:], start=True, stop=True)
    r_sb = work.tile([BS, 1], F32)
    nc.vector.reciprocal(r_sb[:], s_ps[:])

    rb_ps = psum.tile([P, 1], F32)
    nc.tensor.matmul(rb_ps[:], gmat_sb[:], r_sb[:], start=True, stop=True)
    rb_sb = work.tile([P, 1], F32)
    nc.scalar.copy(rb_sb[:], rb_ps[:])

    t_ps = psum.tile([P, P], F32)
    nc.tensor.transpose(t_ps[:], p_all[:], iden[:])
    attn_sb = work.tile([P, P], F32)
    nc.vector.tensor_scalar(
        out=attn_sb[:],
        in0=t_ps[:],
        scalar1=rb_sb[:],
        scalar2=None,
        op0=mybir.AluOpType.mult,
    )
    nc.sync.dma_start(out_ext[:], attn_sb[:])


def build_nc(
    repeat=1,
    use_for_i=False,
    order="b",
    variant="lblock",
    bufs=8,
    do_dma=True,
    do_dve=True,
):
    nc = bacc.Bacc(trn_type="TRN2")

    enc = nc.declare_dram_parameter("enc", [L, BS, H], F32, isOutput=False)
    hid = nc.declare_dram_parameter("hid", [1, BS * H], F32, isOutput=False)
    gmat = nc.declare_dram_parameter("gmat", [BS, P], F32, isOutput=False)
    out_ext = nc.declare_dram_parameter("out", [NC_COLS, P], F32, isOutput=True)

    with tile.TileContext(nc) as tc:
        with (
            tc.tile_pool(name="consts", bufs=1) as consts,
            tc.tile_pool(name="inp", bufs=bufs) as inp,
            tc.tile_pool(name="work", bufs=1) as work,
            tc.tile_pool(name="psum", bufs=1, space="PSUM") as psum,
        ):
            hid_row = consts.tile([1, BS * H], F32)
            nc.sync.dma_start(hid_row[:], hid[:])
            gmat_sb = consts.tile([BS, P], F32)
            nc.sync.dma_start(gmat_sb[:], gmat[:])
            iden = consts.tile([P, P], F32)
            make_identity(nc, iden[:])
            ones = consts.tile([P, 1], F32)
            nc.gpsimd.memset(ones[:], 1.0)

            hidb = consts.tile([P, BS * H], F32)
            nc.gpsimd.partition_broadcast(hidb[:], hid_row[:], P)

            en = work.tile([P, NC_COLS], F32)
            dummy = work.tile([P, 1], F32)
            if repeat == 0 or not do_dve:
                nc.gpsimd.memset(en[:], 0.0)
            fake = None
            if not do_dma:
                fake = consts.tile([P, BS * H], F32)
                nc.gpsimd.memset(fake[:], 0.5)

            def body():
                if variant == "lblock":
                    main_loop_lblock(
                        nc, inp, enc, hidb, en, dummy, do_dma, do_dve, fake
                    )
                else:
                    main_loop(nc, inp, enc, hidb, en, dummy, order)

            if use_for_i and repeat > 1:
                with tc.For_i(0, repeat, 1):
                    body()
            else:
                for _rep in range(repeat):
                    body()

            softmax_out(nc, work, psum, en, gmat_sb, iden, ones, out_ext)

    nc.compile()
    return nc


def make_in_maps(hidden, encoder_outputs):
    hidden = np.ascontiguousarray(np.asarray(hidden, dtype=np.float32))
    enc = np.ascontiguousarray(np.asarray(encoder_outputs, dtype=np.float32))
    assert hidden.shape == (H, B) and enc.shape == (L, B, H)

    gmat = np.zeros((BS, P), np.float32)
    for b in range(BS):
        gmat[b, b * NT : (b + 1) * NT] = 1.0

    in_maps = []
    for c in range(NCORES):
        bsl = slice(c * BS, (c + 1) * BS)
        in_maps.append(
            {
                "enc": np.ascontiguousarray(enc[:, bsl, :]),
                "hid": np.ascontiguousarray(hidden[:, bsl].T.reshape(1, BS * H)),
                "gmat": gmat,
            }
        )
    return in_maps


def _get_nc():
    if "nc" not in _cached:
        _cached["nc"] = build_nc()
    return _cached["nc"]


def kernel(hidden, encoder_outputs, **kwargs):
    in_maps = make_in_maps(hidden, encoder_outputs)
    nc = _get_nc()
    res = run_bass_kernel_spmd(nc, in_maps, core_ids=list(range(NCORES)))
    outs = [res.results[i]["out"].reshape(BS, 1, L) for i in range(NCORES)]
    return np.concatenate(outs, axis=0)

